# revision 25
# baseline (speedup 1.0000x reference)
"""Trainium2 Bass kernel for LiftSplatShoot voxel pooling (segment-sum).

Sharding: 8 cores = (batch b, BEV-grid half h); core owns segments
[h*20000, (h+1)*20000) of batch b.

v3 design (gather-only, no scatter):
  - Segments renumbered into "slots" by descending total count; slot s
    lives at (window w = s//128, partition r = s%128).  Window tile
    counts are unified across the 8 cores so one SPMD program fits all.
  - Per (chunk of <=8 windows, cam): one SWDGE dma_gather pulls the
    cam's points for those windows (slot-sorted, padded per window to
    128-token tiles with the cam's zero row) into SBUF.  Gathers run on
    SWDGE queues 1-3: their descriptor generation executes on dedicated
    Q7 core pairs WITHOUT occupying the Pool engine, so three streams
    generate in parallel (~3x the single-queue rate that bottlenecked
    the previous kernel at 1.33 ms).
  - Each tile is cast to bf16 and multiplied into its window's PSUM
    accumulator: matmul(psum[slot, ch] += onehot[token, slot]^T @
    x[token, ch]) where the one-hot is built on the DVE by comparing an
    uploaded per-token window-local slot id against an iota constant
    (one batched compare per chunk).  PSUM accumulates all cams and all
    tiles of a window; zero-row padding contributes nothing.
  - PSUM -> SBUF -> dense DMA write of slot-major grid rows.  Host
    (index math only) un-permutes slots to segments and assembles the
    output exactly like the reference layout.
"""

import sys

import numpy as np

sys.path.insert(0, "/opt/trn_rl_repo")

# ---- problem constants ----
B, N, D, H, W, C = 4, 6, 41, 16, 44, 64
NX, NY, NZ = 200, 200, 1
CAM_ROWS = D * H * W          # 28864 rows per camera
NSEG_H = NX * NY * NZ // 2    # 20000 segments per core
ZROW = CAM_ROWS               # per-camera zero row index (28864)
PART = 128
N_CORES = 8
MAX_WIN = 8                   # windows per chunk (PSUM: 8*64 f32 = 1 bank)
MAX_TILES = 48                # gathered tiles per chunk (12KB/partition f32)

LAST_RESULTS = None


def _wrap16(lst):
    """int16 token list -> [128, n/16] SBUF wrap (16-partition, replicated)."""
    n = len(lst)
    assert n % 16 == 0
    w = lst.reshape(n // 16, 16).T.astype(np.int16)  # [16, n/16]
    return np.tile(w, (8, 1))


def _host_core(geom_b, h):
    """Per-core slot layout: sorted point lists + window/cam counts."""
    g = geom_b.reshape(N, CAM_ROWS, 3).astype(np.int64)
    tot = np.zeros(NSEG_H, np.int64)
    pts = []
    for c in range(N):
        gx, gy, gz = g[c, :, 0], g[c, :, 1], g[c, :, 2]
        kept = (gx >= 0) & (gx < NX) & (gy >= 0) & (gy < NY) \
            & (gz >= 0) & (gz < NZ)
        seg = gx * (NY * NZ) + gy * NZ + gz
        sel = kept & (seg >= h * NSEG_H) & (seg < (h + 1) * NSEG_H)
        rows = np.nonzero(sel)[0].astype(np.int64)
        sl = (seg[sel] - h * NSEG_H).astype(np.int64)
        pts.append((rows, sl))
        np.add.at(tot, sl, 1)

    order = np.argsort(-tot, kind="stable")
    nnz = int((tot > 0).sum())
    NW = (nnz + PART - 1) // PART
    # balanced (strided) slot assignment: rank i -> window i % NW, so
    # every window gets one segment from each count level and per-(w,c)
    # counts stay just under one 128-token tile.
    slot_of = np.empty(NSEG_H, np.int64)
    ranks = np.arange(NW * PART)
    slot_of[order[:NW * PART]] = (ranks % NW) * PART + ranks // NW
    rest = NSEG_H - NW * PART
    if rest > 0:
        slot_of[order[NW * PART:]] = NW * PART + np.arange(rest)

    rows_c, slots_c = [], []
    wcnt = np.zeros((NW, N), np.int64)
    for c in range(N):
        rows, sl = pts[c]
        slots = slot_of[sl]
        o = np.argsort(slots, kind="stable")
        rows_c.append(rows[o])
        slots_c.append(slots[o])
        w = slots_c[c] // PART
        assert (w < NW).all()
        np.add.at(wcnt, (w, np.full(len(w), c)), 1)

    return dict(NW=NW, slot_of=slot_of, rows_c=rows_c, slots_c=slots_c,
                wcnt=wcnt)


def _unify(plans):
    """Unified window/cam tile counts + chunking (shared by all cores)."""
    NW_u = max(p["NW"] for p in plans)
    T = np.zeros((NW_u, N), np.int64)        # tiles per (window, cam)
    for p in plans:
        t = (p["wcnt"] + PART - 1) // PART
        T[:p["NW"]] = np.maximum(T[:p["NW"]], t)
    T = np.maximum(T, 1)                     # >=1 tile per (w, c)

    chunks = []
    w0 = 0
    while w0 < NW_u:
        nw = 1
        while (w0 + nw < NW_u and nw < MAX_WIN
               and T[w0:w0 + nw + 1].sum() <= MAX_TILES):
            nw += 1
        chunks.append((w0, nw))
        w0 += nw

    # tile layout: chunk-major; within chunk cam-major; within cam
    # window-major.  Record per (chunk) the cam call slices and per-tile
    # (window-local) targets.
    layout = []
    gt = 0
    for (w0, nw) in chunks:
        calls = []      # (cam, t0_global, n_tiles)
        tiles = []      # per tile: (window_local, cam)
        t0c = gt
        for c in range(N):
            n_c = int(T[w0:w0 + nw, c].sum())
            calls.append((c, gt, n_c))
            for wl in range(nw):
                for _ in range(int(T[w0 + wl, c])):
                    tiles.append((wl, c))
            gt += n_c
        layout.append(dict(w0=w0, nw=nw, t0=t0c, calls=calls, tiles=tiles))
    return dict(NW_u=NW_u, T=T, chunks=layout, tot_tiles=gt)


def _core_tokens(p, uni):
    """Per-core gather token array [tot_tiles, 128] + rel [128, tot_tiles]."""
    tot_tiles = uni["tot_tiles"]
    gl = np.full((tot_tiles, PART), ZROW, np.int16)
    rl = np.zeros((PART, tot_tiles), np.float32)
    NW, T = p["NW"], uni["T"]
    # per cam: pointer into sorted lists
    pos = np.zeros(N, np.int64)
    for ch in uni["chunks"]:
        w0, nw = ch["w0"], ch["nw"]
        for (c, t0, n_c) in ch["calls"]:
            rows, slots = p["rows_c"][c], p["slots_c"][c]
            t = t0
            for wl in range(nw):
                w = w0 + wl
                n_t = int(T[w, c])
                if w < NW:
                    # count of this core's tokens in (w, c)
                    k = int(p["wcnt"][w, c])
                else:
                    k = 0
                pz = pos[c]
                assert k <= n_t * PART
                for j in range(n_t):
                    lo = pz + j * PART
                    hi = min(pz + k, lo + PART)
                    if hi > lo:
                        m = hi - lo
                        gl[t + j, :m] = rows[lo:hi].astype(np.int16)
                        rl[:m, t + j] = (slots[lo:hi] - w * PART)
                pos[c] += k
                t += n_t
    return gl, rl


def _build_program(uni):
    from concourse import bacc, mybir, tile
    from concourse.bass import MemorySpace

    nc = bacc.Bacc("TRN2", target_bir_lowering=False, debug=False,
                   num_devices=N_CORES, dynamic_dma_scratch_size=49152,
                   num_swdge_queues=4)
    f32 = mybir.dt.float32
    bf16 = mybir.dt.bfloat16
    i16 = mybir.dt.int16

    tot_tiles = uni["tot_tiles"]
    NW_u = uni["NW_u"]

    xz = [nc.dram_tensor(f"xz{c}", [CAM_ROWS + 1, C], f32,
                         kind="ExternalInput") for c in range(N)]
    gidx_d = nc.dram_tensor("gidx", [PART, tot_tiles * 8], i16,
                            kind="ExternalInput")
    rel_d = nc.dram_tensor("rel", [PART, tot_tiles], f32,
                           kind="ExternalInput")
    iota_d = nc.dram_tensor("iota", [PART, PART], f32,
                            kind="ExternalInput")
    out_d = nc.dram_tensor("out", [NW_u * PART, C], f32,
                           kind="ExternalOutput")

    with tile.TileContext(nc) as tc:
        with (
            tc.tile_pool(name="cons", bufs=1) as cp,
            tc.tile_pool(name="gp", bufs=4) as gp,
            tc.tile_pool(name="xp", bufs=4) as xp,
            tc.tile_pool(name="ep", bufs=3) as ep,
            tc.tile_pool(name="sp", bufs=4) as sp,
            tc.tile_pool(name="pp", bufs=4, space=MemorySpace.PSUM) as pp,
        ):
            n_t0 = len(uni["chunks"][0]["tiles"])
            gidx_sb0 = cp.tile([PART, n_t0 * 8], i16, name="gidx_sb0")
            nc.sync.dma_start(out=gidx_sb0[:], in_=gidx_d[:, :n_t0 * 8])
            gidx_sb = cp.tile([PART, tot_tiles * 8], i16, name="gidx_sb")
            nc.sync.dma_start(out=gidx_sb[:, n_t0 * 8:],
                              in_=gidx_d[:, n_t0 * 8:])
            rel_f = cp.tile([PART, tot_tiles], f32, name="rel_f")
            nc.sync.dma_start(out=rel_f[:], in_=rel_d[:])
            rel_sb = cp.tile([PART, tot_tiles], bf16, name="rel_sb")
            nc.vector.tensor_copy(rel_sb[:], rel_f[:])
            iota_f = cp.tile([PART, PART], f32, name="iota_f")
            nc.sync.dma_start(out=iota_f[:], in_=iota_d[:])
            iota_sb = cp.tile([PART, PART], bf16, name="iota_sb")
            nc.vector.tensor_copy(iota_sb[:], iota_f[:])

            nreg = {}
            for ch in uni["chunks"]:
                for (c, tc0, n_c) in ch["calls"]:
                    if n_c > 0 and n_c * PART not in nreg:
                        nreg[n_c * PART] = nc.gpsimd.to_reg(n_c * PART)

            qn = 0
            for ci, ch in enumerate(uni["chunks"]):
                w0, nw, t0 = ch["w0"], ch["nw"], ch["t0"]
                tiles = len(ch["tiles"])
                buf = gp.tile([PART, MAX_TILES * C], f32, tag="gbuf",
                              name=f"gbuf{ci}")
                for (c, tc0, n_c) in ch["calls"]:
                    if n_c == 0:
                        continue
                    n_in = n_c * PART
                    rel_t = tc0 - t0
                    nc.gpsimd.dma_gather(
                        out_ap=buf[:, rel_t * C:(rel_t + n_c) * C]
                        .rearrange("p (t c) -> p t c", t=n_c, c=C),
                        in_ap=xz[c][:, :],
                        idxs_ap=(gidx_sb0[:, tc0 * 8:(tc0 + n_c) * 8]
                                 if ci == 0 else
                                 gidx_sb[:, tc0 * 8:(tc0 + n_c) * 8]),
                        num_idxs=n_in, num_idxs_reg=nreg[n_in], elem_size=C,
                        single_packet=False,
                        queue_num=(1 + qn % 3) if qn < 12
                        else (1 + qn) % 4)
                    qn += 1

                xbf = xp.tile([PART, MAX_TILES * C], bf16, tag="xbf",
                              name=f"xbf{ci}")
                for (c, tc0, n_c) in ch["calls"]:
                    if n_c == 0:
                        continue
                    rel_t = tc0 - t0
                    nc.scalar.activation(
                        xbf[:, rel_t * C:(rel_t + n_c) * C],
                        buf[:, rel_t * C:(rel_t + n_c) * C],
                        mybir.ActivationFunctionType.Copy)

                eq = ep.tile([PART, MAX_TILES * PART], bf16, tag="eq",
                             name=f"eq{ci}")
                nc.vector.tensor_tensor(
                    out=eq[:, :tiles * PART]
                    .rearrange("p (t i) -> p t i", t=tiles, i=PART),
                    in0=rel_sb[:, t0:t0 + tiles][:, :, None]
                    .broadcast_to([PART, tiles, PART]),
                    in1=iota_sb[:, None, :]
                    .broadcast_to([PART, tiles, PART]),
                    op=mybir.AluOpType.is_equal)

                # NOTE: start=True resets the ENTIRE PSUM bank, so exactly
                # one start per chunk (first matmul) and one stop (last);
                # all windows of the chunk share the bank and accumulate.
                ps = pp.tile([PART, nw * C], f32, tag="ps", name=f"ps{ci}")
                n_mm = len(ch["tiles"])
                for ti, (wl, c) in enumerate(ch["tiles"]):
                    nc.tensor.matmul(
                        ps[:, wl * C:(wl + 1) * C],
                        eq[:, ti * PART:(ti + 1) * PART],
                        xbf[:, ti * C:(ti + 1) * C],
                        start=(ti == 0),
                        stop=(ti == n_mm - 1),
                        skip_group_check=True,
                    )

                stage = sp.tile([PART, MAX_WIN * C], f32, tag="stage",
                                name=f"stage{ci}")
                nc.vector.tensor_copy(stage[:, :nw * C], ps[:])
                nc.sync.dma_start(
                    out=out_d[w0 * PART:(w0 + nw) * PART, :]
                    .rearrange("(w r) c -> r w c", w=nw, r=PART),
                    in_=stage[:, :nw * C]
                    .rearrange("p (w c) -> p w c", w=nw, c=C))

    nc.compile()
    return nc


def _numpy_fallback(x, geom_feats):
    feats = np.asarray(x).reshape(-1, C)
    g = np.asarray(geom_feats).reshape(-1, 3).astype(np.int64)
    npr = feats.shape[0]
    batch_ix = np.repeat(np.arange(B, dtype=np.int64), npr // B)
    kept = (
        (g[:, 0] >= 0) & (g[:, 0] < NX)
        & (g[:, 1] >= 0) & (g[:, 1] < NY)
        & (g[:, 2] >= 0) & (g[:, 2] < NZ)
    )
    feats = np.where(kept[:, None], feats, 0.0)
    seg = batch_ix * (2 * NSEG_H) + g[:, 0] * NY * NZ + g[:, 1] * NZ + g[:, 2]
    seg = np.where(kept, seg, batch_ix * (2 * NSEG_H))
    pooled = np.zeros((B * 2 * NSEG_H, C), np.float32)
    np.add.at(pooled, seg, feats)
    grid = pooled.reshape(B, NX, NY, NZ, C).transpose(0, 4, 3, 1, 2)
    return np.ascontiguousarray(grid.reshape(B, C * NZ, NX, NY))


def kernel(x, geom_feats):
    from concourse import bass_utils

    x = np.ascontiguousarray(np.asarray(x, dtype=np.float32))
    geom = np.asarray(geom_feats)

    plans = []
    for b in range(B):
        for h in range(2):
            plans.append(_host_core(geom[b], h) | {"b": b, "h": h})
    uni = _unify(plans)

    nc = _build_program(uni)

    zero_row = np.zeros((1, C), np.float32)
    in_maps = []
    for p in plans:
        gl, rl = _core_tokens(p, uni)
        xb = x[p["b"]].reshape(N, CAM_ROWS, C)
        m = {f"xz{c}": np.concatenate([xb[c], zero_row], axis=0)
             for c in range(N)}
        m["gidx"] = _wrap16(gl.reshape(-1))
        m["rel"] = rl
        m["iota"] = np.tile(np.arange(PART, dtype=np.float32), (PART, 1))
        in_maps.append(m)

    res = bass_utils.run_bass_kernel_spmd(
        nc, in_maps, core_ids=list(range(N_CORES)), trace=True)
    global LAST_RESULTS
    LAST_RESULTS = res

    out = np.zeros((B, C * NZ, NX, NY), np.float32)
    for p, r in zip(plans, res.results):
        rows = np.asarray(r["out"], np.float32)   # [NW_u*128, C] slot-major
        full = np.zeros((NSEG_H, C), np.float32)
        slot = p["slot_of"]
        cov = slot < p["NW"] * PART
        full[cov] = rows[slot[cov]]
        grid = full.reshape(NX // 2, NY, C)
        out[p["b"], :, p["h"] * (NX // 2):(p["h"] + 1) * (NX // 2), :] = \
            grid.transpose(2, 0, 1)
    return out


# revision 26
# speedup vs baseline: 1.2386x; 1.2386x over previous
"""Trainium2 Bass kernel for LiftSplatShoot voxel pooling (segment-sum).

Sharding: 8 cores = (batch b, BEV-grid half h); core owns segments
[h*20000, (h+1)*20000) of batch b.

v3 design (gather-only, no scatter):
  - Segments renumbered into "slots" by descending total count; slot s
    lives at (window w = s//128, partition r = s%128).  Window tile
    counts are unified across the 8 cores so one SPMD program fits all.
  - Per (chunk of <=8 windows, cam): one SWDGE dma_gather pulls the
    cam's points for those windows (slot-sorted, padded per window to
    128-token tiles with the cam's zero row) into SBUF.  Gathers run on
    SWDGE queues 1-3: their descriptor generation executes on dedicated
    Q7 core pairs WITHOUT occupying the Pool engine, so three streams
    generate in parallel (~3x the single-queue rate that bottlenecked
    the previous kernel at 1.33 ms).
  - Each tile is cast to bf16 and multiplied into its window's PSUM
    accumulator: matmul(psum[slot, ch] += onehot[token, slot]^T @
    x[token, ch]) where the one-hot is built on the DVE by comparing an
    uploaded per-token window-local slot id against an iota constant
    (one batched compare per chunk).  PSUM accumulates all cams and all
    tiles of a window; zero-row padding contributes nothing.
  - PSUM -> SBUF -> dense DMA write of slot-major grid rows.  Host
    (index math only) un-permutes slots to segments and assembles the
    output exactly like the reference layout.
"""

import sys

import numpy as np

sys.path.insert(0, "/opt/trn_rl_repo")

# ---- problem constants ----
B, N, D, H, W, C = 4, 6, 41, 16, 44, 64
NX, NY, NZ = 200, 200, 1
CAM_ROWS = D * H * W          # 28864 rows per camera
NSEG_H = NX * NY * NZ // 2    # 20000 segments per core
ZROW = CAM_ROWS               # per-camera zero row index (28864)
PART = 128
N_CORES = 8
MAX_WIN = 8                   # windows per chunk (PSUM: 8*64 f32 = 1 bank)
MAX_TILES = 48                # gathered tiles per chunk (12KB/partition f32)

LAST_RESULTS = None


def _wrap16(lst):
    """int16 token list -> [128, n/16] SBUF wrap (16-partition, replicated)."""
    n = len(lst)
    assert n % 16 == 0
    w = lst.reshape(n // 16, 16).T.astype(np.int16)  # [16, n/16]
    return np.tile(w, (8, 1))


def _host_core(geom_b, h):
    """Per-core slot layout: sorted point lists + window/cam counts."""
    g = geom_b.reshape(N, CAM_ROWS, 3).astype(np.int64)
    tot = np.zeros(NSEG_H, np.int64)
    pts = []
    for c in range(N):
        gx, gy, gz = g[c, :, 0], g[c, :, 1], g[c, :, 2]
        kept = (gx >= 0) & (gx < NX) & (gy >= 0) & (gy < NY) \
            & (gz >= 0) & (gz < NZ)
        seg = gx * (NY * NZ) + gy * NZ + gz
        sel = kept & (seg >= h * NSEG_H) & (seg < (h + 1) * NSEG_H)
        rows = np.nonzero(sel)[0].astype(np.int64)
        sl = (seg[sel] - h * NSEG_H).astype(np.int64)
        pts.append((rows, sl))
        np.add.at(tot, sl, 1)

    order = np.argsort(-tot, kind="stable")
    nnz = int((tot > 0).sum())
    NW = (nnz + PART - 1) // PART
    # balanced (strided) slot assignment: rank i -> window i % NW, so
    # every window gets one segment from each count level and per-(w,c)
    # counts stay just under one 128-token tile.
    slot_of = np.empty(NSEG_H, np.int64)
    ranks = np.arange(NW * PART)
    slot_of[order[:NW * PART]] = (ranks % NW) * PART + ranks // NW
    rest = NSEG_H - NW * PART
    if rest > 0:
        slot_of[order[NW * PART:]] = NW * PART + np.arange(rest)

    rows_c, slots_c = [], []
    wcnt = np.zeros((NW, N), np.int64)
    for c in range(N):
        rows, sl = pts[c]
        slots = slot_of[sl]
        o = np.argsort(slots, kind="stable")
        rows_c.append(rows[o])
        slots_c.append(slots[o])
        w = slots_c[c] // PART
        assert (w < NW).all()
        np.add.at(wcnt, (w, np.full(len(w), c)), 1)

    return dict(NW=NW, slot_of=slot_of, rows_c=rows_c, slots_c=slots_c,
                wcnt=wcnt)


def _unify(plans):
    """Unified window/cam tile counts + chunking (shared by all cores)."""
    NW_u = max(p["NW"] for p in plans)
    T = np.zeros((NW_u, N), np.int64)        # tiles per (window, cam)
    for p in plans:
        t = (p["wcnt"] + PART - 1) // PART
        T[:p["NW"]] = np.maximum(T[:p["NW"]], t)
    T = np.maximum(T, 1)                     # >=1 tile per (w, c)

    chunks = []
    w0 = 0
    while w0 < NW_u:
        nw = 1
        while (w0 + nw < NW_u and nw < MAX_WIN
               and T[w0:w0 + nw + 1].sum() <= MAX_TILES):
            nw += 1
        chunks.append((w0, nw))
        w0 += nw

    # tile layout: chunk-major; within chunk cam-major; within cam
    # window-major.  Record per (chunk) the cam call slices and per-tile
    # (window-local) targets.
    layout = []
    gt = 0
    for (w0, nw) in chunks:
        calls = []      # (cam, t0_global, n_tiles)
        tiles = []      # per tile: (window_local, cam)
        t0c = gt
        for c in range(N):
            n_c = int(T[w0:w0 + nw, c].sum())
            calls.append((c, gt, n_c))
            for wl in range(nw):
                for _ in range(int(T[w0 + wl, c])):
                    tiles.append((wl, c))
            gt += n_c
        layout.append(dict(w0=w0, nw=nw, t0=t0c, calls=calls, tiles=tiles))
    return dict(NW_u=NW_u, T=T, chunks=layout, tot_tiles=gt)


def _core_tokens(p, uni):
    """Per-core gather token array [tot_tiles, 128] + rel [128, tot_tiles]."""
    tot_tiles = uni["tot_tiles"]
    gl = np.full((tot_tiles, PART), ZROW, np.int16)
    rl = np.zeros((PART, tot_tiles), np.float32)
    NW, T = p["NW"], uni["T"]
    # per cam: pointer into sorted lists
    pos = np.zeros(N, np.int64)
    for ch in uni["chunks"]:
        w0, nw = ch["w0"], ch["nw"]
        for (c, t0, n_c) in ch["calls"]:
            rows, slots = p["rows_c"][c], p["slots_c"][c]
            t = t0
            for wl in range(nw):
                w = w0 + wl
                n_t = int(T[w, c])
                if w < NW:
                    # count of this core's tokens in (w, c)
                    k = int(p["wcnt"][w, c])
                else:
                    k = 0
                pz = pos[c]
                assert k <= n_t * PART
                for j in range(n_t):
                    lo = pz + j * PART
                    hi = min(pz + k, lo + PART)
                    if hi > lo:
                        m = hi - lo
                        gl[t + j, :m] = rows[lo:hi].astype(np.int16)
                        rl[:m, t + j] = (slots[lo:hi] - w * PART)
                pos[c] += k
                t += n_t
    return gl, rl


def _build_program(uni):
    from concourse import bacc, mybir, tile
    from concourse.bass import MemorySpace

    nc = bacc.Bacc("TRN2", target_bir_lowering=False, debug=False,
                   num_devices=N_CORES, dynamic_dma_scratch_size=49152,
                   num_swdge_queues=4)
    f32 = mybir.dt.float32
    bf16 = mybir.dt.bfloat16
    i16 = mybir.dt.int16

    tot_tiles = uni["tot_tiles"]
    NW_u = uni["NW_u"]

    xz = [nc.dram_tensor(f"xz{c}", [CAM_ROWS + 1, C], f32,
                         kind="ExternalInput") for c in range(N)]
    gidx_d = nc.dram_tensor("gidx", [PART, tot_tiles * 8], i16,
                            kind="ExternalInput")
    rel_d = nc.dram_tensor("rel", [PART, tot_tiles], f32,
                           kind="ExternalInput")
    iota_d = nc.dram_tensor("iota", [PART, PART], f32,
                            kind="ExternalInput")
    out_d = nc.dram_tensor("out", [NW_u * PART, C], f32,
                           kind="ExternalOutput")

    with tile.TileContext(nc) as tc:
        with (
            tc.tile_pool(name="cons", bufs=1) as cp,
            tc.tile_pool(name="gp", bufs=4) as gp,
            tc.tile_pool(name="xp", bufs=4) as xp,
            tc.tile_pool(name="ep", bufs=3) as ep,
            tc.tile_pool(name="sp", bufs=4) as sp,
            tc.tile_pool(name="pp", bufs=4, space=MemorySpace.PSUM) as pp,
        ):
            n_t0 = len(uni["chunks"][0]["tiles"])
            gidx_sb0 = cp.tile([PART, n_t0 * 8], i16, name="gidx_sb0")
            nc.sync.dma_start(out=gidx_sb0[:], in_=gidx_d[:, :n_t0 * 8])
            gidx_sb = cp.tile([PART, tot_tiles * 8], i16, name="gidx_sb")
            nc.sync.dma_start(out=gidx_sb[:, n_t0 * 8:],
                              in_=gidx_d[:, n_t0 * 8:])
            rel_f = cp.tile([PART, tot_tiles], f32, name="rel_f")
            nc.sync.dma_start(out=rel_f[:], in_=rel_d[:])
            rel_sb = cp.tile([PART, tot_tiles], bf16, name="rel_sb")
            nc.vector.tensor_copy(rel_sb[:], rel_f[:])
            iota_f = cp.tile([PART, PART], f32, name="iota_f")
            nc.sync.dma_start(out=iota_f[:], in_=iota_d[:])
            iota_sb = cp.tile([PART, PART], bf16, name="iota_sb")
            nc.vector.tensor_copy(iota_sb[:], iota_f[:])

            nreg = {}
            for ch in uni["chunks"]:
                for (c, tc0, n_c) in ch["calls"]:
                    if n_c > 0 and n_c * PART not in nreg:
                        nreg[n_c * PART] = nc.gpsimd.to_reg(n_c * PART)

            qn = 0
            for ci, ch in enumerate(uni["chunks"]):
                w0, nw, t0 = ch["w0"], ch["nw"], ch["t0"]
                tiles = len(ch["tiles"])
                buf = gp.tile([PART, MAX_TILES * C], f32, tag="gbuf",
                              name=f"gbuf{ci}")
                for (c, tc0, n_c) in ch["calls"]:
                    if n_c == 0:
                        continue
                    n_in = n_c * PART
                    rel_t = tc0 - t0
                    nc.gpsimd.dma_gather(
                        out_ap=buf[:, rel_t * C:(rel_t + n_c) * C]
                        .rearrange("p (t c) -> p t c", t=n_c, c=C),
                        in_ap=xz[c][:, :],
                        idxs_ap=(gidx_sb0[:, tc0 * 8:(tc0 + n_c) * 8]
                                 if ci == 0 else
                                 gidx_sb[:, tc0 * 8:(tc0 + n_c) * 8]),
                        num_idxs=n_in, num_idxs_reg=nreg[n_in], elem_size=C,
                        single_packet=False, queue_num=(1 + qn) % 4)
                    qn += 1

                xbf = xp.tile([PART, MAX_TILES * C], bf16, tag="xbf",
                              name=f"xbf{ci}")
                for (c, tc0, n_c) in ch["calls"]:
                    if n_c == 0:
                        continue
                    rel_t = tc0 - t0
                    nc.scalar.activation(
                        xbf[:, rel_t * C:(rel_t + n_c) * C],
                        buf[:, rel_t * C:(rel_t + n_c) * C],
                        mybir.ActivationFunctionType.Copy)

                eq = ep.tile([PART, MAX_TILES * PART], bf16, tag="eq",
                             name=f"eq{ci}")
                nc.vector.tensor_tensor(
                    out=eq[:, :tiles * PART]
                    .rearrange("p (t i) -> p t i", t=tiles, i=PART),
                    in0=rel_sb[:, t0:t0 + tiles][:, :, None]
                    .broadcast_to([PART, tiles, PART]),
                    in1=iota_sb[:, None, :]
                    .broadcast_to([PART, tiles, PART]),
                    op=mybir.AluOpType.is_equal)

                # NOTE: start=True resets the ENTIRE PSUM bank, so exactly
                # one start per chunk (first matmul) and one stop (last);
                # all windows of the chunk share the bank and accumulate.
                ps = pp.tile([PART, nw * C], f32, tag="ps", name=f"ps{ci}")
                n_mm = len(ch["tiles"])
                for ti, (wl, c) in enumerate(ch["tiles"]):
                    nc.tensor.matmul(
                        ps[:, wl * C:(wl + 1) * C],
                        eq[:, ti * PART:(ti + 1) * PART],
                        xbf[:, ti * C:(ti + 1) * C],
                        start=(ti == 0),
                        stop=(ti == n_mm - 1),
                        skip_group_check=True,
                    )

                stage = sp.tile([PART, MAX_WIN * C], f32, tag="stage",
                                name=f"stage{ci}")
                nc.vector.tensor_copy(stage[:, :nw * C], ps[:])
                nc.sync.dma_start(
                    out=out_d[w0 * PART:(w0 + nw) * PART, :]
                    .rearrange("(w r) c -> r w c", w=nw, r=PART),
                    in_=stage[:, :nw * C]
                    .rearrange("p (w c) -> p w c", w=nw, c=C))

    nc.compile()
    return nc


def _numpy_fallback(x, geom_feats):
    feats = np.asarray(x).reshape(-1, C)
    g = np.asarray(geom_feats).reshape(-1, 3).astype(np.int64)
    npr = feats.shape[0]
    batch_ix = np.repeat(np.arange(B, dtype=np.int64), npr // B)
    kept = (
        (g[:, 0] >= 0) & (g[:, 0] < NX)
        & (g[:, 1] >= 0) & (g[:, 1] < NY)
        & (g[:, 2] >= 0) & (g[:, 2] < NZ)
    )
    feats = np.where(kept[:, None], feats, 0.0)
    seg = batch_ix * (2 * NSEG_H) + g[:, 0] * NY * NZ + g[:, 1] * NZ + g[:, 2]
    seg = np.where(kept, seg, batch_ix * (2 * NSEG_H))
    pooled = np.zeros((B * 2 * NSEG_H, C), np.float32)
    np.add.at(pooled, seg, feats)
    grid = pooled.reshape(B, NX, NY, NZ, C).transpose(0, 4, 3, 1, 2)
    return np.ascontiguousarray(grid.reshape(B, C * NZ, NX, NY))


def kernel(x, geom_feats):
    from concourse import bass_utils

    x = np.ascontiguousarray(np.asarray(x, dtype=np.float32))
    geom = np.asarray(geom_feats)

    plans = []
    for b in range(B):
        for h in range(2):
            plans.append(_host_core(geom[b], h) | {"b": b, "h": h})
    uni = _unify(plans)

    nc = _build_program(uni)

    zero_row = np.zeros((1, C), np.float32)
    in_maps = []
    for p in plans:
        gl, rl = _core_tokens(p, uni)
        xb = x[p["b"]].reshape(N, CAM_ROWS, C)
        m = {f"xz{c}": np.concatenate([xb[c], zero_row], axis=0)
             for c in range(N)}
        m["gidx"] = _wrap16(gl.reshape(-1))
        m["rel"] = rl
        m["iota"] = np.tile(np.arange(PART, dtype=np.float32), (PART, 1))
        in_maps.append(m)

    res = bass_utils.run_bass_kernel_spmd(
        nc, in_maps, core_ids=list(range(N_CORES)), trace=True)
    global LAST_RESULTS
    LAST_RESULTS = res

    out = np.zeros((B, C * NZ, NX, NY), np.float32)
    for p, r in zip(plans, res.results):
        rows = np.asarray(r["out"], np.float32)   # [NW_u*128, C] slot-major
        full = np.zeros((NSEG_H, C), np.float32)
        slot = p["slot_of"]
        cov = slot < p["NW"] * PART
        full[cov] = rows[slot[cov]]
        grid = full.reshape(NX // 2, NY, C)
        out[p["b"], :, p["h"] * (NX // 2):(p["h"] + 1) * (NX // 2), :] = \
            grid.transpose(2, 0, 1)
    return out
